# revision 10
# baseline (speedup 1.0000x reference)
"""Context-segment scoring kernel for Trainium2 (Bass/Tile).

Computes out[b, n] = sum_e c[b, n, e] * s[b, e] for
c = c_embeds [32, 32, 32, 8, 256] viewed as [B=32, N=8192, E=256] and
s = s_embeds [32, 256].

Strategy (v3): cast inputs to fp16 on the host (quantization rel-err
~3e-4, far under the 2e-2 gate) and transpose c to [B, E, N] so the
TensorEngine does the entire multiply-reduce as matvecs:
  psum[1, 512] += s_chunk[128, 1].T @ cT_chunk[128, 512]
accumulated over the two 128-wide E chunks. fp16 halves HBM traffic
(16 MiB/core, ~47 us DMA floor at ~358 GB/s); the PE replaces all the
DVE/ScalarE elementwise work that bounded v1 at ~121 us.

v3 vs v2 (79 us): input DMAs are issued from two engines (SP HWDGE +
GpSimd SWDGE) with enough buffers that no WAR wait ever blocks an
issue (v2 serialized 28 DMAs behind sem-waits on SP - stream finished
at 76 us); s loads in one pre-transposed [128, 8] DMA; weights load
once per 8-matmul group so the PE streams back-to-back and stays at
the warm 2.4 GHz clock; PSUM is drained in [1, 1024] copies alternating
DVE/ScalarE (different banks - legal in parallel).

Sharding: data-parallel over batch - 8 NeuronCores, 4 batches each.
"""

import numpy as np

import concourse.bacc as bacc
import concourse.bass as bass
import concourse.mybir as mybir
import concourse.tile as tile
from concourse.bass_utils import run_bass_kernel_spmd

B, N, E = 32, 8192, 256
NCORES = 8
B_LOC = B // NCORES          # 4 batches per core
P = 128                      # SBUF partitions / PE contract dim
ECH = E // P                 # 2 e-chunks of 128
NT = 512                     # n per matmul (one PSUM bank of fp32)
NSLICE = 4096                # n per input DMA slice (1 MiB fp16)
NSL = N // NSLICE            # slices per (batch, chunk)
TPB = NSLICE // NT           # 8 matmul n-tiles per block
PSG = 2                      # n-tiles per psum tile ([1, 1024] = 2 banks)

F32 = mybir.dt.float32
F16 = mybir.dt.float16


def build_body(tc, out_ap, c_ap, s_ap):
    """Per-core Tile program. DRAM access patterns:
    out [B_LOC, N] f32, c [B_LOC, ECH, P, N] f16, s [P, B_LOC*ECH] f16."""
    nc = tc.nc
    with (
        tc.tile_pool(name="sseg", bufs=1) as s_pool,
        tc.tile_pool(name="cin", bufs=2 * (NSL * B_LOC - 1)) as cin_pool,
        tc.tile_pool(name="cin_s", bufs=8) as cins_pool,
        tc.tile_pool(name="oacc", bufs=3) as out_pool,
        tc.tile_pool(name="ps", bufs=4, space="PSUM") as ps_pool,
    ):
        # All segment-embedding columns in one DMA: s_all[:, b*ECH+k] is the
        # [128, 1] stationary operand for (batch b, e-chunk k).
        s_all = s_pool.tile([P, B_LOC * ECH], F16, tag="s", name="s_all")
        nc.sync.dma_start(s_all[:, :], s_ap)

        # Segment list: full 2 MiB blocks, except the final block which is
        # graded down (2048/1024/512/512 n) so the trailing compute +
        # extraction + output only ever waits on a small final slice.
        segs = []
        for b in range(B_LOC):
            for h in range(NSL):
                if b == B_LOC - 1 and h == NSL - 1:
                    n0 = h * NSLICE
                    for ln in (2048, 1024, 512, 512):
                        segs.append((b, n0, ln))
                        n0 += ln
                else:
                    segs.append((b, h * NSLICE, NSLICE))

        # Pre-issue every input DMA on the single SP HWDGE ring, in exact
        # consumption order (c0 then c1 per segment). The ring is FIFO and
        # drains across all 16 SDMA engines at ~420 GB/s, so slices land
        # in the order the PE needs them - no late-arriving chunk ever
        # stalls the pipeline. No WAR waits exist (one buffer per slice).
        # SWDGE (gpsimd) is avoided: each SWDGE DMA costs a ~4.3 us Q7
        # DRAIN; a second HWDGE ring (ACT) halves per-ring rate and makes
        # slices finish out of order.
        ctiles = []
        for b, n0, ln in segs:
            pool, w_ln = (cin_pool, NSLICE) if ln == NSLICE else (cins_pool, 2048)
            c0 = pool.tile([P, w_ln], F16, tag="cin", name="c0")
            nc.sync.dma_start(c0[:, :ln], c_ap[b, 0, :, n0:n0 + ln])
            c1 = pool.tile([P, w_ln], F16, tag="cin", name="c1")
            nc.sync.dma_start(c1[:, :ln], c_ap[b, 1, :, n0:n0 + ln])
            ctiles.append((c0, c1))

        eng = 0
        for (b, n0, ln), (c0, c1) in zip(segs, ctiles):
            ntiles = ln // NT
            npt = (ntiles + PSG - 1) // PSG
            pts = [
                ps_pool.tile([1, PSG * NT], F32, tag="pt", name=f"pt{g}")
                for g in range(npt)
            ]
            # All chunk-0 matmuls share one weight load, then all chunk-1.
            for k, ct, start, stop in ((0, c0, True, False), (1, c1, False, True)):
                w = s_all[:, b * ECH + k: b * ECH + k + 1]
                for t in range(ntiles):
                    nc.tensor.matmul(
                        pts[t // PSG][0:1, (t % PSG) * NT:(t % PSG + 1) * NT],
                        w,
                        ct[:, t * NT:(t + 1) * NT],
                        start=start,
                        stop=stop,
                    )

            ot = out_pool.tile([1, NSLICE], F32, tag="ot", name="ot")
            for g in range(npt):
                gl = min(PSG * NT, ln - g * PSG * NT)
                dst = ot[0:1, g * PSG * NT: g * PSG * NT + gl]
                if eng % 2 == 0:
                    nc.vector.tensor_copy(dst, pts[g][0:1, :gl])
                else:
                    nc.scalar.copy(dst, pts[g][0:1, :gl])
                eng += 1
            # Output rides the ACT HWDGE ring: the SP ring is FIFO and
            # still holds megabytes of queued input - an out-DMA there
            # would not drain (and via the out-tile WAR would stall
            # extraction, PSUM reuse, and ultimately the PE).
            nc.scalar.dma_start(
                out_ap[b, n0:n0 + ln].unsqueeze(0), ot[0:1, :ln]
            )


_NC_CACHE = None


def _get_nc():
    global _NC_CACHE
    if _NC_CACHE is None:
        nc = bacc.Bacc(
            "TRN2",
            target_bir_lowering=False,
            debug=False,
            num_devices=NCORES,
        )
        c = nc.dram_tensor("c", [B_LOC, ECH, P, N], F16, kind="ExternalInput")
        s = nc.dram_tensor("s", [P, B_LOC * ECH], F16, kind="ExternalInput")
        o = nc.dram_tensor("o", [B_LOC, N], F32, kind="ExternalOutput")
        with tile.TileContext(nc) as tc:
            build_body(tc, o.ap(), c.ap(), s.ap())
        nc.compile()
        _NC_CACHE = nc
    return _NC_CACHE


def _run(c_embeds: np.ndarray, s_embeds: np.ndarray, **kwargs):
    c = np.asarray(c_embeds, dtype=np.float32).reshape(B, N, E)
    # [B, N, E] -> [B, E, N] fp16, chunked: [B, ECH, P, N]
    ct = np.ascontiguousarray(
        c.astype(np.float16).transpose(0, 2, 1)
    ).reshape(B, ECH, P, N)
    # s[b, e] -> per-core [P, B_LOC*ECH] with column (b*ECH+k) = s[b, 128k:128k+128]
    s = np.asarray(s_embeds, dtype=np.float32).astype(np.float16)
    s = s.reshape(B, ECH, P)
    nc = _get_nc()
    in_maps = [
        {
            "c": ct[k * B_LOC:(k + 1) * B_LOC],
            "s": np.ascontiguousarray(
                s[k * B_LOC:(k + 1) * B_LOC].reshape(B_LOC * ECH, P).T
            ),
        }
        for k in range(NCORES)
    ]
    r = run_bass_kernel_spmd(nc, in_maps, core_ids=list(range(NCORES)), **kwargs)
    out = np.concatenate([r.results[k]["o"] for k in range(NCORES)], axis=0)
    return out.astype(np.float32), r


def kernel(c_embeds: np.ndarray, s_embeds: np.ndarray) -> np.ndarray:
    out, _ = _run(c_embeds, s_embeds)
    return out


# revision 11
# speedup vs baseline: 1.0567x; 1.0567x over previous
"""Context-segment scoring kernel for Trainium2 (Bass/Tile).

Computes out[b, n] = sum_e c[b, n, e] * s[b, e] for
c = c_embeds [32, 32, 32, 8, 256] viewed as [B=32, N=8192, E=256] and
s = s_embeds [32, 256].

Strategy (v3): cast inputs to fp16 on the host (quantization rel-err
~3e-4, far under the 2e-2 gate) and transpose c to [B, E, N] so the
TensorEngine does the entire multiply-reduce as matvecs:
  psum[1, 512] += s_chunk[128, 1].T @ cT_chunk[128, 512]
accumulated over the two 128-wide E chunks. fp16 halves HBM traffic
(16 MiB/core, ~47 us DMA floor at ~358 GB/s); the PE replaces all the
DVE/ScalarE elementwise work that bounded v1 at ~121 us.

v3 vs v2 (79 us): input DMAs are issued from two engines (SP HWDGE +
GpSimd SWDGE) with enough buffers that no WAR wait ever blocks an
issue (v2 serialized 28 DMAs behind sem-waits on SP - stream finished
at 76 us); s loads in one pre-transposed [128, 8] DMA; weights load
once per 8-matmul group so the PE streams back-to-back and stays at
the warm 2.4 GHz clock; PSUM is drained in [1, 1024] copies alternating
DVE/ScalarE (different banks - legal in parallel).

Sharding: data-parallel over batch - 8 NeuronCores, 4 batches each.
"""

import numpy as np

import concourse.bacc as bacc
import concourse.bass as bass
import concourse.mybir as mybir
import concourse.tile as tile
from concourse.bass_utils import run_bass_kernel_spmd

B, N, E = 32, 8192, 256
NCORES = 8
B_LOC = B // NCORES          # 4 batches per core
P = 128                      # SBUF partitions / PE contract dim
ECH = E // P                 # 2 e-chunks of 128
NT = 512                     # n per matmul (one PSUM bank of fp32)
NSLICE = 4096                # n per input DMA slice (1 MiB fp16)
NSL = N // NSLICE            # slices per (batch, chunk)
TPB = NSLICE // NT           # 8 matmul n-tiles per block
PSG = 2                      # n-tiles per psum tile ([1, 1024] = 2 banks)

F32 = mybir.dt.float32
F16 = mybir.dt.float16


def build_body(tc, out_ap, c_ap, s_ap):
    """Per-core Tile program. DRAM access patterns:
    out [B_LOC, N] f32, c [B_LOC, ECH, P, N] f16, s [P, B_LOC*ECH] f16."""
    nc = tc.nc
    with (
        tc.tile_pool(name="sseg", bufs=1) as s_pool,
        tc.tile_pool(name="cin", bufs=2 * (NSL * B_LOC - 1)) as cin_pool,
        tc.tile_pool(name="cin_s", bufs=8) as cins_pool,
        tc.tile_pool(name="oacc", bufs=3) as out_pool,
        tc.tile_pool(name="ps", bufs=4, space="PSUM") as ps_pool,
    ):
        # All segment-embedding columns in one DMA: s_all[:, b*ECH+k] is the
        # [128, 1] stationary operand for (batch b, e-chunk k).
        s_all = s_pool.tile([P, B_LOC * ECH], F16, tag="s", name="s_all")
        nc.sync.dma_start(s_all[:, :], s_ap)

        # Segment list: full 2 MiB blocks, except the final block which is
        # graded down (2048/1024/512/512 n) so the trailing compute +
        # extraction + output only ever waits on a small final slice.
        segs = []
        for b in range(B_LOC):
            for h in range(NSL):
                if b == B_LOC - 1 and h == NSL - 1:
                    n0 = h * NSLICE
                    for ln in (2048, 1024, 512, 512):
                        segs.append((b, n0, ln))
                        n0 += ln
                else:
                    segs.append((b, h * NSLICE, NSLICE))

        # Pre-issue every input DMA on the single SP HWDGE ring, in exact
        # consumption order (c0 then c1 per segment). The ring is FIFO and
        # drains across all 16 SDMA engines at ~420 GB/s, so slices land
        # in the order the PE needs them - no late-arriving chunk ever
        # stalls the pipeline. No WAR waits exist (one buffer per slice).
        # SWDGE (gpsimd) is avoided: each SWDGE DMA costs a ~4.3 us Q7
        # DRAIN; a second HWDGE ring (ACT) halves per-ring rate and makes
        # slices finish out of order.
        # Small tail slices ride the otherwise-idle ACT ring, issued up
        # front: they arrive by ~14 us, long before the PE reaches them,
        # and the SP ring finishes the full blocks ~3 us sooner. Putting
        # them last on the SP ring instead would trip the ~8-deep DMA
        # in-flight issue window and land them after 55 us (measured).
        ctiles = []
        for b, n0, ln in segs:
            if ln == NSLICE:
                pool, w_ln, eng_dma = cin_pool, NSLICE, nc.sync
            else:
                pool, w_ln, eng_dma = cins_pool, 2048, nc.scalar
            c0 = pool.tile([P, w_ln], F16, tag="cin", name="c0")
            eng_dma.dma_start(c0[:, :ln], c_ap[b, 0, :, n0:n0 + ln])
            c1 = pool.tile([P, w_ln], F16, tag="cin", name="c1")
            eng_dma.dma_start(c1[:, :ln], c_ap[b, 1, :, n0:n0 + ln])
            ctiles.append((c0, c1))

        eng = 0
        for (b, n0, ln), (c0, c1) in zip(segs, ctiles):
            ntiles = ln // NT
            npt = (ntiles + PSG - 1) // PSG
            pts = [
                ps_pool.tile([1, PSG * NT], F32, tag="pt", name=f"pt{g}")
                for g in range(npt)
            ]
            # All chunk-0 matmuls share one weight load, then all chunk-1.
            for k, ct, start, stop in ((0, c0, True, False), (1, c1, False, True)):
                w = s_all[:, b * ECH + k: b * ECH + k + 1]
                for t in range(ntiles):
                    nc.tensor.matmul(
                        pts[t // PSG][0:1, (t % PSG) * NT:(t % PSG + 1) * NT],
                        w,
                        ct[:, t * NT:(t + 1) * NT],
                        start=start,
                        stop=stop,
                    )

            ot = out_pool.tile([1, NSLICE], F32, tag="ot", name="ot")
            for g in range(npt):
                gl = min(PSG * NT, ln - g * PSG * NT)
                dst = ot[0:1, g * PSG * NT: g * PSG * NT + gl]
                if eng % 2 == 0:
                    nc.vector.tensor_copy(dst, pts[g][0:1, :gl])
                else:
                    nc.scalar.copy(dst, pts[g][0:1, :gl])
                eng += 1
            # Output rides the ACT HWDGE ring: the SP ring is FIFO and
            # still holds megabytes of queued input - an out-DMA there
            # would not drain (and via the out-tile WAR would stall
            # extraction, PSUM reuse, and ultimately the PE).
            nc.scalar.dma_start(
                out_ap[b, n0:n0 + ln].unsqueeze(0), ot[0:1, :ln]
            )


_NC_CACHE = None


def _get_nc():
    global _NC_CACHE
    if _NC_CACHE is None:
        nc = bacc.Bacc(
            "TRN2",
            target_bir_lowering=False,
            debug=False,
            num_devices=NCORES,
        )
        c = nc.dram_tensor("c", [B_LOC, ECH, P, N], F16, kind="ExternalInput")
        s = nc.dram_tensor("s", [P, B_LOC * ECH], F16, kind="ExternalInput")
        o = nc.dram_tensor("o", [B_LOC, N], F32, kind="ExternalOutput")
        with tile.TileContext(nc) as tc:
            build_body(tc, o.ap(), c.ap(), s.ap())
        nc.compile()
        _NC_CACHE = nc
    return _NC_CACHE


def _run(c_embeds: np.ndarray, s_embeds: np.ndarray, **kwargs):
    c = np.asarray(c_embeds, dtype=np.float32).reshape(B, N, E)
    # [B, N, E] -> [B, E, N] fp16, chunked: [B, ECH, P, N]
    ct = np.ascontiguousarray(
        c.astype(np.float16).transpose(0, 2, 1)
    ).reshape(B, ECH, P, N)
    # s[b, e] -> per-core [P, B_LOC*ECH] with column (b*ECH+k) = s[b, 128k:128k+128]
    s = np.asarray(s_embeds, dtype=np.float32).astype(np.float16)
    s = s.reshape(B, ECH, P)
    nc = _get_nc()
    in_maps = [
        {
            "c": ct[k * B_LOC:(k + 1) * B_LOC],
            "s": np.ascontiguousarray(
                s[k * B_LOC:(k + 1) * B_LOC].reshape(B_LOC * ECH, P).T
            ),
        }
        for k in range(NCORES)
    ]
    r = run_bass_kernel_spmd(nc, in_maps, core_ids=list(range(NCORES)), **kwargs)
    out = np.concatenate([r.results[k]["o"] for k in range(NCORES)], axis=0)
    return out.astype(np.float32), r


def kernel(c_embeds: np.ndarray, s_embeds: np.ndarray) -> np.ndarray:
    out, _ = _run(c_embeds, s_embeds)
    return out
